# revision 10
# baseline (speedup 1.0000x reference)
"""Trainium2 Bass kernel for nn_BERT_LSTM_CRF (embedding MixedOp + Linear +
bidirectional LSTM + output projection), SPMD over 8 NeuronCores.

Sharding: cores 0-3 forward LSTM / cores 4-7 reverse LSTM (reverse is run as a
forward scan over host-flipped sequences); within each direction group the
batch (32) is sharded 4 ways (8 rows per core). Embedding tables are
replicated; each core gathers only the rows for its own 4096 tokens.

Per-core pipeline (all cores run the identical program, only data differs):
  P0  softmax(arch_params) on device; scaled identity matrices; gate bias
      d = bih + bhh + Wih @ b1.
  P1  for each chunk of 512 tokens: indirect-DMA gather of table rows
      -> PE transpose (f32r) -> x^T (bf16); W1 matmul -> xin^T (bf16);
      Wih matmul (+bias) -> xg^T -> DRAM (bf16).
  P2  512-step LSTM recurrence, gates-on-partitions layout, gate order
      [g,i,f,o]: xg injected into PSUM via a scaled-identity matmul
      (start=True), Whh^T matmuls (fp8e4, x16 scale) accumulate on top;
      ACT reads PSUM directly with scale=1/16; i/f/g chain overlaps the
      o-gate matmuls via split PSUM banks.
  P3  Wout half-projection of h^T history -> partial output [22, 4096].

Host reassembles: out[b,s,:] = fwd_part + rev_part (flipped).
"""

import contextlib
import ctypes
import os
import sys
import types

sys.path.insert(0, "/opt/trn_rl_repo")

import numpy as np

import concourse.bacc as bacc
import concourse.bass as bass
import concourse.mybir as mybir
import concourse.tile as tile
from concourse.bass_utils import run_bass_kernel_spmd

F32 = mybir.dt.float32
F32R = mybir.dt.float32r
BF16 = mybir.dt.bfloat16
FP8 = mybir.dt.float8e4
I32 = mybir.dt.int32
AF = mybir.ActivationFunctionType
ALU = mybir.AluOpType

P = 128
DE = 256          # embedding dim per table
NE = 3            # number of tables
EMB = 512         # after W1
HID = 512
G4 = 4 * HID      # 2048 gate dim
TAGP2 = 22
B_LOC = 8         # batch rows per core
N_CORES = 8
SC = 16.0         # fp8 weight scale (ACT un-scales); 1.0 in bf16 mode
# gate order [g, i, f, o] (pytorch blocks are [i, f, g, o])
GPERM = (2, 0, 1, 3)

LAST_EXEC_NS = None


# --------------------------------------------------------------------------
# NTFF profiling shim (antenv.axon_hooks is missing from this image).
def _install_ntff_shim():
    if "antenv.axon_hooks" in sys.modules:
        return

    def _make_hook():
        try:
            lib = ctypes.CDLL("/opt/axon/libaxon_pjrt.so")
        except OSError:
            return None
        if not hasattr(lib, "axon_start_nrt_profile"):
            return None
        lib.axon_start_nrt_profile.argtypes = [
            ctypes.POINTER(ctypes.c_int64),
            ctypes.c_size_t,
        ]
        lib.axon_start_nrt_profile.restype = ctypes.c_int64
        lib.axon_stop_nrt_profile.argtypes = [ctypes.c_char_p]
        lib.axon_stop_nrt_profile.restype = ctypes.c_int64

        @contextlib.contextmanager
        def _hook(output_dir, device_ids):
            import jax

            jax.devices()
            if device_ids:
                ids = (ctypes.c_int64 * len(device_ids))(*device_ids)
                rc = lib.axon_start_nrt_profile(ids, len(device_ids))
            else:
                rc = lib.axon_start_nrt_profile(None, 0)
            if rc != 0:
                raise RuntimeError(f"axon_start_nrt_profile rc={rc}")
            try:
                yield
            finally:
                n = lib.axon_stop_nrt_profile(str(output_dir).encode())
                if n < 0:
                    raise RuntimeError(f"axon_stop_nrt_profile rc={n}")

        return _hook

    mod = types.ModuleType("antenv.axon_hooks")
    mod.get_axon_ntff_profile_hook = _make_hook
    sys.modules["antenv.axon_hooks"] = mod


_install_ntff_shim()


# --------------------------------------------------------------------------
def build_nc(S, V, whh_fp8=True):
    """Build the per-core Bass program. S = sequence length, V = vocab."""
    n_tok = B_LOC * S                    # tokens per core
    n_tile = n_tok // P                  # 128-token tiles
    CH_TOK = 512 if n_tok >= 512 else n_tok   # tokens per P1 chunk
    n_ch = n_tok // CH_TOK               # P1 chunks
    ch_tile = CH_TOK // P                # token-tiles per chunk (4)
    n_gj = n_tile * NE                   # gather calls
    WDT = FP8 if whh_fp8 else BF16
    sc = SC if whh_fp8 else 1.0

    nc = bacc.Bacc("TRN2", target_bir_lowering=False, debug=False,
                   num_devices=N_CORES)

    tables = nc.dram_tensor("tables", [NE * V, DE], F32, kind="ExternalInput")
    gidx_in = nc.dram_tensor("gidx", [P, n_gj], I32, kind="ExternalInput")
    arch_in = nc.dram_tensor("arch", [1, NE], F32, kind="ExternalInput")
    w1_in = nc.dram_tensor("w1", [P, 6 * EMB], BF16, kind="ExternalInput")
    wih_in = nc.dram_tensor("wihT", [P, 4 * G4], BF16, kind="ExternalInput")
    whh_in = nc.dram_tensor("whhT", [P, 4 * G4], WDT, kind="ExternalInput")
    wout_in = nc.dram_tensor("wout", [P, 4 * TAGP2], BF16, kind="ExternalInput")
    b1_in = nc.dram_tensor("b1c", [P, 4], BF16, kind="ExternalInput")
    bih_in = nc.dram_tensor("bihg", [P, 16], F32, kind="ExternalInput")
    bhh_in = nc.dram_tensor("bhhg", [P, 16], F32, kind="ExternalInput")
    bout_in = nc.dram_tensor("boutc", [TAGP2, 1], F32, kind="ExternalInput")
    ident_in = nc.dram_tensor("identc", [P, P], BF16, kind="ExternalInput")
    identr_in = nc.dram_tensor("identr", [P, P], F32, kind="ExternalInput")
    outp = nc.dram_tensor("outp", [TAGP2, n_tok], F32, kind="ExternalOutput")

    # xg^T staging in DRAM: row = gate row (16 tiles x 128), col = s*8+b
    xgT = nc.dram_tensor("xgT", [16 * P, S * B_LOC], BF16, kind="Internal")

    with tile.TileContext(nc) as tc:
        ctx = contextlib.ExitStack()
        with ctx:
            constp = ctx.enter_context(tc.tile_pool(name="constp", bufs=1))
            wper = ctx.enter_context(tc.tile_pool(name="wper", bufs=1))
            psum0_cm = tc.tile_pool(name="psum0", bufs=1, space="PSUM")
            psum0 = psum0_cm.__enter__()

            # ---------------- P0: constants -------------------------------
            gidx_sb = wper.tile([P, n_gj], I32)
            nc.sync.dma_start(out=gidx_sb[:], in_=gidx_in.ap())
            whh_sb = wper.tile([P, 4 * G4], WDT)
            nc.sync.dma_start(out=whh_sb[:], in_=whh_in.ap())
            wout_sb = wper.tile([P, 4 * TAGP2], BF16)
            nc.sync.dma_start(out=wout_sb[:], in_=wout_in.ap())
            bout_sb = wper.tile([TAGP2, 1], F32)
            nc.sync.dma_start(out=bout_sb[:], in_=bout_in.ap())

            # softmax(arch) broadcast to all partitions
            arow = constp.tile([1, NE], F32)
            nc.sync.dma_start(out=arow[:], in_=arch_in.ap())
            erow = constp.tile([1, NE], F32)
            nc.scalar.activation(erow[:], arow[:], AF.Exp)
            srow = constp.tile([1, 1], F32)
            nc.vector.tensor_reduce(out=srow[:], in_=erow[:],
                                    axis=mybir.AxisListType.X, op=ALU.add)
            ones_r = constp.tile([1, P], F32)
            nc.vector.memset(ones_r[:], 1.0)
            rrow = constp.tile([1, 1], F32)
            nc.vector.reciprocal(out=rrow[:], in_=srow[:])
            pe_b = psum0.tile([P, NE], F32, space="PSUM", tag="pe_b")
            nc.tensor.matmul(pe_b[:], lhsT=ones_r[:], rhs=erow[:],
                             start=True, stop=True)
            ps_b = psum0.tile([P, 1], F32, space="PSUM", tag="ps_b")
            nc.tensor.matmul(ps_b[:], lhsT=ones_r[:], rhs=rrow[:],
                             start=True, stop=True)
            ssb = constp.tile([P, 1], F32)
            nc.vector.tensor_copy(out=ssb[:], in_=ps_b[:])
            wbc = constp.tile([P, NE], F32)
            nc.vector.tensor_tensor(out=wbc[:], in0=pe_b[:],
                                    in1=ssb[:].to_broadcast([P, NE]),
                                    op=ALU.mult)

            ident_sc_t = wper.tile([P, P], BF16)     # xg inject (sc * I)
            nc.sync.dma_start(out=ident_sc_t[:], in_=ident_in.ap())
            ident_sc = ident_sc_t[:]
            ident_tr_t = wper.tile([P, P], F32)      # transpose helper
            nc.sync.dma_start(out=ident_tr_t[:], in_=identr_in.ap())
            ident_tr = ident_tr_t[:]

            # gate bias dcol[p, m] = bih + bhh + (Wih @ b1), layout (p, gt)
            b1_sb = constp.tile([P, 4], BF16)
            nc.sync.dma_start(out=b1_sb[:], in_=b1_in.ap())
            bih_sb = constp.tile([P, 16], F32)
            nc.sync.dma_start(out=bih_sb[:], in_=bih_in.ap())
            bhh_sb = constp.tile([P, 16], F32)
            nc.sync.dma_start(out=bhh_sb[:], in_=bhh_in.ap())
            dcol = wper.tile([P, 16], F32)
            nc.vector.tensor_add(out=dcol[:], in0=bih_sb[:], in1=bhh_sb[:])

            wih_sb = wper.tile([P, 4 * G4], BF16)
            nc.sync.dma_start(out=wih_sb[:], in_=wih_in.ap())
            for m in range(16):
                pd = psum0.tile([P, 1], F32, space="PSUM", tag="pd")
                for k in range(4):
                    nc.tensor.matmul(
                        pd[:],
                        lhsT=wih_sb[:, k * G4 + m * P:k * G4 + (m + 1) * P],
                        rhs=b1_sb[:, k:k + 1],
                        start=(k == 0), stop=(k == 3))
                nc.vector.tensor_add(out=dcol[:, m:m + 1], in0=dcol[:, m:m + 1],
                                     in1=pd[:])
            psum0_cm.__exit__(None, None, None)

            # ---------------- P1: gather -> x^T -> xin^T -> xg^T ----------
            with tc.tile_pool(name="p1w", bufs=1) as p1w, \
                 tc.tile_pool(name="p1g", bufs=3) as p1g, \
                 tc.tile_pool(name="p1t", bufs=2) as p1t, \
                 tc.tile_pool(name="p1e", bufs=4) as p1e, \
                 tc.tile_pool(name="psum_t", bufs=2, space="PSUM") as psum_t, \
                 tc.tile_pool(name="psum_x", bufs=2, space="PSUM") as psum_x, \
                 tc.tile_pool(name="psum_g", bufs=2, space="PSUM") as psum_g:

                w1_sb = p1w.tile([P, 6 * EMB], BF16)
                nc.sync.dma_start(out=w1_sb[:], in_=w1_in.ap())
                # fold softmax(arch) scale into W1 rows (k-tile k has table
                # index k//2 throughout: 256-row blocks, 128-row tiles)
                for k in range(6):
                    nc.vector.tensor_scalar_mul(
                        w1_sb[:, k * EMB:(k + 1) * EMB],
                        w1_sb[:, k * EMB:(k + 1) * EMB],
                        wbc[:, k // 2:k // 2 + 1])

                for ci in range(n_ch):
                    xT = p1t.tile([P, 6 * CH_TOK], BF16, tag="xT")
                    for ti in range(ch_tile):
                        xg_t = p1g.tile([P, NE * DE], F32, tag="xg_t")
                        for e in range(NE):
                            j = (ci * ch_tile + ti) * NE + e
                            nc.gpsimd.indirect_dma_start(
                                out=xg_t[:, e * DE:(e + 1) * DE],
                                out_offset=None,
                                in_=tables.ap(),
                                in_offset=bass.IndirectOffsetOnAxis(
                                    ap=gidx_sb[:, j:j + 1], axis=0),
                            )
                        for fc in range(6):
                            pt = psum_t.tile([P, P], F32, space="PSUM",
                                             tag="pt")
                            nc.tensor.transpose(
                                out=pt[:],
                                in_=xg_t[:, fc * P:(fc + 1) * P],
                                identity=ident_tr)
                            nc.vector.tensor_copy(
                                out=xT[:, fc * CH_TOK + ti * P:
                                       fc * CH_TOK + (ti + 1) * P],
                                in_=pt[:])

                    xinT = p1t.tile([P, 4 * CH_TOK], BF16, tag="xinT")
                    for m in range(4):
                        px = psum_x.tile([P, CH_TOK], F32, space="PSUM",
                                         tag="px")
                        for k in range(6):
                            nc.tensor.matmul(
                                px[:],
                                lhsT=w1_sb[:, k * EMB + m * P:
                                           k * EMB + (m + 1) * P],
                                rhs=xT[:, k * CH_TOK:(k + 1) * CH_TOK],
                                start=(k == 0), stop=(k == 5))
                        nc.vector.tensor_copy(
                            out=xinT[:, m * CH_TOK:(m + 1) * CH_TOK], in_=px[:])

                    for m in range(16):
                        pg = psum_g.tile([P, CH_TOK], F32, space="PSUM",
                                         tag="pg")
                        for k in range(4):
                            nc.tensor.matmul(
                                pg[:],
                                lhsT=wih_sb[:, k * G4 + m * P:
                                            k * G4 + (m + 1) * P],
                                rhs=xinT[:, k * CH_TOK:(k + 1) * CH_TOK],
                                start=(k == 0), stop=(k == 3))
                        ev = p1e.tile([P, CH_TOK], BF16, tag="ev")
                        nc.vector.tensor_scalar_add(ev[:], pg[:],
                                                    dcol[:, m:m + 1])
                        nc.sync.dma_start(
                            out=xgT.ap()[m * P:(m + 1) * P,
                                         ci * CH_TOK:(ci + 1) * CH_TOK],
                            in_=ev[:])

            # ---------------- P2: LSTM recurrence -------------------------
            # gate tiles (new order): 0-3 g, 4-7 i, 8-11 f, 12-15 o
            with tc.tile_pool(name="hTp", bufs=1) as hTp, \
                 tc.tile_pool(name="stp", bufs=4) as stp, \
                 tc.tile_pool(name="pgg", bufs=2, space="PSUM") as pgg, \
                 tc.tile_pool(name="pif", bufs=2, space="PSUM") as pif, \
                 tc.tile_pool(name="pog", bufs=2, space="PSUM") as pog:

                hT = hTp.tile([P, 4 * n_tok], BF16)
                c_sb = hTp.tile([P, HID // 16], F32)   # [128, 32]
                nc.vector.memset(c_sb[:], 0.0)

                BL = B_LOC
                HB = 4 * BL  # 32 cols per gate type
                inv = 1.0 / sc
                for t in range(S):
                    xgt = stp.tile([P, 16 * BL], BF16, tag="xgt")
                    nc.sync.dma_start(
                        out=xgt[:].rearrange("g (gt b) -> g gt b", gt=16),
                        in_=xgT.ap()[:, t * BL:(t + 1) * BL].rearrange(
                            "(gt g) b -> g gt b", g=P))
                    pr_gg = pgg.tile([P, HB], F32, space="PSUM", tag="pr_gg")
                    pr_if = pif.tile([P, 2 * HB], F32, space="PSUM",
                                     tag="pr_if")
                    pr_o = pog.tile([P, HB], F32, space="PSUM", tag="pr_o")
                    # inject sc*xg via identity matmuls (set has_written)
                    nc.tensor.matmul(pr_gg[:], lhsT=ident_sc,
                                     rhs=xgt[:, 0:HB],
                                     start=True, stop=(t == 0),
                                     skip_group_check=True)
                    nc.tensor.matmul(pr_if[:], lhsT=ident_sc,
                                     rhs=xgt[:, HB:3 * HB],
                                     start=True, stop=(t == 0),
                                     skip_group_check=True)
                    nc.tensor.matmul(pr_o[:], lhsT=ident_sc,
                                     rhs=xgt[:, 3 * HB:4 * HB],
                                     start=True, stop=(t == 0),
                                     skip_group_check=True)
                    if t > 0:
                        for gt in range(16):
                            if gt < 4:
                                dst = pr_gg[:, gt * BL:(gt + 1) * BL]
                            elif gt < 12:
                                dst = pr_if[:, (gt - 4) * BL:(gt - 3) * BL]
                            else:
                                dst = pr_o[:, (gt - 12) * BL:(gt - 11) * BL]
                            for kt in range(4):
                                rh = hT[:, kt * n_tok + (t - 1) * BL:
                                        kt * n_tok + t * BL]
                                nc.tensor.matmul(
                                    dst,
                                    lhsT=whh_sb[:, kt * G4 + gt * P:
                                                kt * G4 + (gt + 1) * P],
                                    rhs=rh,
                                    start=False, stop=(kt == 3),
                                    skip_group_check=True)

                    tg = stp.tile([P, HB], F32, tag="tg")
                    nc.scalar.activation(tg[:], pr_gg[:], AF.Tanh,
                                         scale=inv)
                    sif = stp.tile([P, 2 * HB], F32, tag="sif")
                    nc.scalar.activation(sif[:], pr_if[:],
                                         AF.Sigmoid, scale=inv)
                    so = stp.tile([P, HB], F32, tag="so")
                    nc.scalar.activation(so[:], pr_o[:], AF.Sigmoid,
                                         scale=inv)
                    fc_ = stp.tile([P, HB], F32, tag="fc_")
                    nc.vector.tensor_tensor(out=fc_[:], in0=sif[:, HB:2 * HB],
                                            in1=c_sb[:], op=ALU.mult)
                    ig_ = stp.tile([P, HB], F32, tag="ig_")
                    nc.vector.tensor_tensor(out=ig_[:], in0=sif[:, 0:HB],
                                            in1=tg[:], op=ALU.mult)
                    nc.vector.tensor_add(out=c_sb[:], in0=fc_[:], in1=ig_[:])
                    tc_ = stp.tile([P, HB], F32, tag="tc_")
                    nc.scalar.activation(tc_[:], c_sb[:], AF.Tanh)
                    nc.vector.tensor_tensor(
                        out=hT[:].rearrange("g (kt n) -> g kt n", kt=4)
                            [:, :, t * BL:(t + 1) * BL],
                        in0=so[:].rearrange("g (kt b) -> g kt b", kt=4),
                        in1=tc_[:].rearrange("g (kt b) -> g kt b", kt=4),
                        op=ALU.mult)

                # ------------- P3: Wout partial ---------------------------
                with tc.tile_pool(name="p3", bufs=2) as p3, \
                     tc.tile_pool(name="psum_o", bufs=2, space="PSUM") as psum_o:
                    oT = p3.tile([TAGP2, n_tok], F32, tag="oT")
                    CH_O = CH_TOK
                    for ci in range(n_tok // CH_O):
                        po = psum_o.tile([TAGP2, CH_O], F32, space="PSUM",
                                         tag="po")
                        for kt in range(4):
                            nc.tensor.matmul(
                                po[:],
                                lhsT=wout_sb[:, kt * TAGP2:(kt + 1) * TAGP2],
                                rhs=hT[:, kt * n_tok + ci * CH_O:
                                       kt * n_tok + (ci + 1) * CH_O],
                                start=(kt == 0), stop=(kt == 3))
                        nc.vector.tensor_scalar_add(
                            oT[:, ci * CH_O:(ci + 1) * CH_O], po[:],
                            bout_sb[:, 0:1])
                    nc.sync.dma_start(out=outp.ap(), in_=oT[:])

    nc.compile()
    return nc


# --------------------------------------------------------------------------
_NC_CACHE = {}


def _get_nc(S, V, whh_fp8=True):
    key = (S, V, whh_fp8)
    if key not in _NC_CACHE:
        _NC_CACHE[key] = build_nc(S, V, whh_fp8)
    return _NC_CACHE[key]


def _gate_perm(a, axis=0):
    """Permute pytorch gate blocks [i,f,g,o] -> [g,i,f,o] along axis."""
    blocks = np.split(np.asarray(a), 4, axis=axis)
    return np.concatenate([blocks[g] for g in GPERM], axis=axis)


def _prep_core_inputs(c, token_ids, tables_flat, arch_params, w1, b1,
                      wih_f, whh_f, bih_f, bhh_f, wih_r, whh_r, bih_r, bhh_r,
                      wout, bout, S, V, whh_fp8):
    import ml_dtypes
    d, g = divmod(c, 4)
    ids = token_ids[g * B_LOC:(g + 1) * B_LOC, :]
    if d == 1:
        ids = ids[:, ::-1]
    flat = ids.T.reshape(-1).astype(np.int64)      # s-major [S*B]
    n_tile = flat.shape[0] // P
    base = flat.reshape(n_tile, P)
    gidx = (base[:, :, None] + (np.arange(NE) * V)[None, None, :])
    gidx = gidx.transpose(1, 0, 2).reshape(P, n_tile * NE).astype(np.int32)

    wih = _gate_perm(wih_f if d == 0 else wih_r)
    whh = _gate_perm(whh_f if d == 0 else whh_r)
    bih = _gate_perm(bih_f if d == 0 else bih_r)
    bhh = _gate_perm(bhh_f if d == 0 else bhh_r)

    wihT = np.ascontiguousarray(wih.T)             # [512, 2048]
    whhT = np.ascontiguousarray(whh.T)

    def ktile(a, nk, f):
        # [nk*128, f] -> [128, nk*f] with (k) tiles side by side
        return np.ascontiguousarray(
            a.reshape(nk, P, f).transpose(1, 0, 2).reshape(P, nk * f))

    if whh_fp8:
        whh_host = np.clip(ktile(whhT, 4, G4) * SC, -240.0, 240.0).astype(
            ml_dtypes.float8_e4m3)
    else:
        whh_host = ktile(whhT, 4, G4).astype(ml_dtypes.bfloat16)

    bf = ml_dtypes.bfloat16
    eye = np.eye(P, dtype=np.float32)
    return {
        "tables": tables_flat,
        "identc": ((SC if whh_fp8 else 1.0) * eye).astype(bf),
        "identr": eye,
        "gidx": gidx,
        "arch": arch_params.reshape(1, NE).astype(np.float32),
        "w1": ktile(w1, 6, EMB).astype(bf),
        "wihT": ktile(wihT, 4, G4).astype(bf),
        "whhT": whh_host,
        "wout": ktile(wout[d * HID:(d + 1) * HID, :], 4, TAGP2).astype(bf),
        "b1c": np.ascontiguousarray(b1.reshape(4, P).T).astype(bf),
        "bihg": np.ascontiguousarray(bih.reshape(16, P).T).astype(np.float32),
        "bhhg": np.ascontiguousarray(bhh.reshape(16, P).T).astype(np.float32),
        "boutc": (bout.reshape(TAGP2, 1).astype(np.float32) if d == 0
                  else np.zeros((TAGP2, 1), np.float32)),
    }


def run_cores(token_ids, emb_tables, arch_params, W1, b1,
              Wih_f, Whh_f, bih_f, bhh_f, Wih_r, Whh_r, bih_r, bhh_r,
              Wout, bout, *, whh_fp8=True, trace=False):
    global LAST_EXEC_NS
    B, S = token_ids.shape
    V = emb_tables.shape[1]
    assert B == 32 and emb_tables.shape[0] == NE and emb_tables.shape[2] == DE

    import time as _time
    _t0 = _time.time()
    nc = _get_nc(S, V, whh_fp8)
    _t1 = _time.time()
    tables_flat = np.ascontiguousarray(
        np.asarray(emb_tables, dtype=np.float32).reshape(NE * V, DE))

    args = (np.asarray(token_ids), tables_flat, np.asarray(arch_params),
            np.asarray(W1), np.asarray(b1),
            np.asarray(Wih_f), np.asarray(Whh_f), np.asarray(bih_f),
            np.asarray(bhh_f),
            np.asarray(Wih_r), np.asarray(Whh_r), np.asarray(bih_r),
            np.asarray(bhh_r), np.asarray(Wout), np.asarray(bout))
    in_maps = [
        _prep_core_inputs(c, *args, S, V, whh_fp8) for c in range(N_CORES)
    ]
    _t2 = _time.time()
    res = run_bass_kernel_spmd(nc, in_maps, list(range(N_CORES)), trace=trace)
    LAST_EXEC_NS = res.exec_time_ns
    if os.environ.get("KERNEL_VERBOSE", "0") == "1":
        print(f"[kernel] build {_t1-_t0:.1f}s prep {_t2-_t1:.1f}s "
              f"run {_time.time()-_t2:.1f}s exec_ns={LAST_EXEC_NS}",
              flush=True)

    out = np.zeros((B, S, TAGP2), dtype=np.float32)
    for c in range(N_CORES):
        d, g = divmod(c, 4)
        part = res.results[c]["outp"]                      # [22, S*B_LOC]
        part = np.asarray(part).T.reshape(S, B_LOC, TAGP2)
        if d == 1:
            part = part[::-1]
        out[g * B_LOC:(g + 1) * B_LOC] += part.transpose(1, 0, 2)
    return out


def kernel(token_ids, emb_tables, arch_params, W1, b1,
           Wih_f, Whh_f, bih_f, bhh_f,
           Wih_r, Whh_r, bih_r, bhh_r,
           Wout, bout):
    return run_cores(
        token_ids, emb_tables, arch_params, W1, b1,
        Wih_f, Whh_f, bih_f, bhh_f, Wih_r, Whh_r, bih_r, bhh_r, Wout, bout,
        whh_fp8=os.environ.get("KERNEL_WHH_FP8", "1") == "1",
        trace=os.environ.get("KERNEL_TRACE", "0") == "1",
    )


# revision 12
# speedup vs baseline: 1.2305x; 1.2305x over previous
"""Trainium2 Bass kernel for nn_BERT_LSTM_CRF (embedding MixedOp + Linear +
bidirectional LSTM + output projection), SPMD over 8 NeuronCores.

Sharding: cores 0-3 forward LSTM / cores 4-7 reverse LSTM (reverse is run as a
forward scan over host-flipped sequences); within each direction group the
batch (32) is sharded 4 ways (8 rows per core). Embedding tables are
replicated; each core gathers only the rows for its own 4096 tokens.

Per-core pipeline (all cores run the identical program, only data differs):
  P0  softmax(arch_params) on device; scaled identity matrices; gate bias
      d = bih + bhh + Wih @ b1.
  P1  for each chunk of 512 tokens: indirect-DMA gather of table rows
      -> PE transpose (f32r) -> x^T (bf16); W1 matmul -> xin^T (bf16);
      Wih matmul (+bias) -> xg^T -> DRAM (bf16).
  P2  512-step LSTM recurrence, gates-on-partitions layout, gate order
      [g,i,f,o]: xg injected into PSUM via a scaled-identity matmul
      (start=True), Whh^T matmuls (fp8e4, x16 scale) accumulate on top;
      ACT reads PSUM directly with scale=1/16; i/f/g chain overlaps the
      o-gate matmuls via split PSUM banks.
  P3  Wout half-projection of h^T history -> partial output [22, 4096].

Host reassembles: out[b,s,:] = fwd_part + rev_part (flipped).
"""

import contextlib
import ctypes
import os
import sys
import types

sys.path.insert(0, "/opt/trn_rl_repo")

import numpy as np

import concourse.bacc as bacc
import concourse.bass as bass
import concourse.mybir as mybir
import concourse.tile as tile
from concourse.bass_utils import run_bass_kernel_spmd

F32 = mybir.dt.float32
F32R = mybir.dt.float32r
BF16 = mybir.dt.bfloat16
FP8 = mybir.dt.float8e4
I32 = mybir.dt.int32
AF = mybir.ActivationFunctionType
ALU = mybir.AluOpType

P = 128
DE = 256          # embedding dim per table
NE = 3            # number of tables
EMB = 512         # after W1
HID = 512
G4 = 4 * HID      # 2048 gate dim
TAGP2 = 22
B_LOC = 8         # batch rows per core
N_CORES = 8
SC = 16.0         # fp8 weight scale (ACT un-scales); 1.0 in bf16 mode
# gate order [g, f, i, o] (pytorch blocks are [i, f, g, o])
GPERM = (2, 1, 0, 3)

LAST_EXEC_NS = None


# --------------------------------------------------------------------------
# NTFF profiling shim (antenv.axon_hooks is missing from this image).
def _install_ntff_shim():
    if "antenv.axon_hooks" in sys.modules:
        return

    def _make_hook():
        try:
            lib = ctypes.CDLL("/opt/axon/libaxon_pjrt.so")
        except OSError:
            return None
        if not hasattr(lib, "axon_start_nrt_profile"):
            return None
        lib.axon_start_nrt_profile.argtypes = [
            ctypes.POINTER(ctypes.c_int64),
            ctypes.c_size_t,
        ]
        lib.axon_start_nrt_profile.restype = ctypes.c_int64
        lib.axon_stop_nrt_profile.argtypes = [ctypes.c_char_p]
        lib.axon_stop_nrt_profile.restype = ctypes.c_int64

        @contextlib.contextmanager
        def _hook(output_dir, device_ids):
            import jax

            jax.devices()
            if device_ids:
                ids = (ctypes.c_int64 * len(device_ids))(*device_ids)
                rc = lib.axon_start_nrt_profile(ids, len(device_ids))
            else:
                rc = lib.axon_start_nrt_profile(None, 0)
            if rc != 0:
                raise RuntimeError(f"axon_start_nrt_profile rc={rc}")
            try:
                yield
            finally:
                n = lib.axon_stop_nrt_profile(str(output_dir).encode())
                if n < 0:
                    raise RuntimeError(f"axon_stop_nrt_profile rc={n}")

        return _hook

    mod = types.ModuleType("antenv.axon_hooks")
    mod.get_axon_ntff_profile_hook = _make_hook
    sys.modules["antenv.axon_hooks"] = mod


_install_ntff_shim()


# --------------------------------------------------------------------------
def build_nc(S, V, whh_fp8=True):
    """Build the per-core Bass program. S = sequence length, V = vocab."""
    n_tok = B_LOC * S                    # tokens per core
    n_tile = n_tok // P                  # 128-token tiles
    CH_TOK = 512 if n_tok >= 512 else n_tok   # tokens per P1 chunk
    n_ch = n_tok // CH_TOK               # P1 chunks
    ch_tile = CH_TOK // P                # token-tiles per chunk (4)
    n_gj = n_tile * NE                   # gather calls
    WDT = FP8 if whh_fp8 else BF16
    sc = SC if whh_fp8 else 1.0

    nc = bacc.Bacc("TRN2", target_bir_lowering=False, debug=False,
                   num_devices=N_CORES)

    tables = nc.dram_tensor("tables", [NE * V, DE], F32, kind="ExternalInput")
    gidx_in = nc.dram_tensor("gidx", [P, n_gj], I32, kind="ExternalInput")
    arch_in = nc.dram_tensor("arch", [1, NE], F32, kind="ExternalInput")
    w1_in = nc.dram_tensor("w1", [P, 6 * EMB], BF16, kind="ExternalInput")
    wih_in = nc.dram_tensor("wihT", [P, 4 * G4], BF16, kind="ExternalInput")
    whh_in = nc.dram_tensor("whhT", [P, 4 * G4], WDT, kind="ExternalInput")
    wout_in = nc.dram_tensor("wout", [P, 4 * TAGP2], BF16, kind="ExternalInput")
    b1_in = nc.dram_tensor("b1c", [P, 4], BF16, kind="ExternalInput")
    bih_in = nc.dram_tensor("bihg", [P, 16], F32, kind="ExternalInput")
    bhh_in = nc.dram_tensor("bhhg", [P, 16], F32, kind="ExternalInput")
    bout_in = nc.dram_tensor("boutc", [TAGP2, 1], F32, kind="ExternalInput")
    ident_in = nc.dram_tensor("identc", [P, P], BF16, kind="ExternalInput")
    identr_in = nc.dram_tensor("identr", [P, P], F32, kind="ExternalInput")
    outp = nc.dram_tensor("outp", [TAGP2, n_tok], F32, kind="ExternalOutput")

    # xg^T staging in DRAM: row = gate row (16 tiles x 128), col = s*8+b
    xgT = nc.dram_tensor("xgT", [16 * P, S * B_LOC], BF16, kind="Internal")

    with tile.TileContext(nc) as tc:
        ctx = contextlib.ExitStack()
        with ctx:
            constp = ctx.enter_context(tc.tile_pool(name="constp", bufs=1))
            wper = ctx.enter_context(tc.tile_pool(name="wper", bufs=1))
            psum0_cm = tc.tile_pool(name="psum0", bufs=1, space="PSUM")
            psum0 = psum0_cm.__enter__()

            # ---------------- P0: constants -------------------------------
            gidx_sb = wper.tile([P, n_gj], I32)
            nc.sync.dma_start(out=gidx_sb[:], in_=gidx_in.ap())
            whh_sb = wper.tile([P, 4 * G4], WDT)
            nc.sync.dma_start(out=whh_sb[:], in_=whh_in.ap())
            wout_sb = wper.tile([P, 4 * TAGP2], BF16)
            nc.sync.dma_start(out=wout_sb[:], in_=wout_in.ap())
            bout_sb = wper.tile([TAGP2, 1], F32)
            nc.sync.dma_start(out=bout_sb[:], in_=bout_in.ap())

            # softmax(arch) broadcast to all partitions
            arow = constp.tile([1, NE], F32)
            nc.sync.dma_start(out=arow[:], in_=arch_in.ap())
            erow = constp.tile([1, NE], F32)
            nc.scalar.activation(erow[:], arow[:], AF.Exp)
            srow = constp.tile([1, 1], F32)
            nc.vector.tensor_reduce(out=srow[:], in_=erow[:],
                                    axis=mybir.AxisListType.X, op=ALU.add)
            ones_r = constp.tile([1, P], F32)
            nc.vector.memset(ones_r[:], 1.0)
            rrow = constp.tile([1, 1], F32)
            nc.vector.reciprocal(out=rrow[:], in_=srow[:])
            pe_b = psum0.tile([P, NE], F32, space="PSUM", tag="pe_b")
            nc.tensor.matmul(pe_b[:], lhsT=ones_r[:], rhs=erow[:],
                             start=True, stop=True)
            ps_b = psum0.tile([P, 1], F32, space="PSUM", tag="ps_b")
            nc.tensor.matmul(ps_b[:], lhsT=ones_r[:], rhs=rrow[:],
                             start=True, stop=True)
            ssb = constp.tile([P, 1], F32)
            nc.vector.tensor_copy(out=ssb[:], in_=ps_b[:])
            wbc = constp.tile([P, NE], F32)
            nc.vector.tensor_tensor(out=wbc[:], in0=pe_b[:],
                                    in1=ssb[:].to_broadcast([P, NE]),
                                    op=ALU.mult)

            ident_sc_t = wper.tile([P, P], BF16)     # xg inject (sc * I)
            nc.sync.dma_start(out=ident_sc_t[:], in_=ident_in.ap())
            ident_sc = ident_sc_t[:]
            ident_tr_t = wper.tile([P, P], F32)      # transpose helper
            nc.sync.dma_start(out=ident_tr_t[:], in_=identr_in.ap())
            ident_tr = ident_tr_t[:]

            # gate bias dcol[p, m] = bih + bhh + (Wih @ b1), layout (p, gt)
            b1_sb = constp.tile([P, 4], BF16)
            nc.sync.dma_start(out=b1_sb[:], in_=b1_in.ap())
            bih_sb = constp.tile([P, 16], F32)
            nc.sync.dma_start(out=bih_sb[:], in_=bih_in.ap())
            bhh_sb = constp.tile([P, 16], F32)
            nc.sync.dma_start(out=bhh_sb[:], in_=bhh_in.ap())
            dcol = wper.tile([P, 16], F32)
            nc.vector.tensor_add(out=dcol[:], in0=bih_sb[:], in1=bhh_sb[:])

            wih_sb = wper.tile([P, 4 * G4], BF16)
            nc.sync.dma_start(out=wih_sb[:], in_=wih_in.ap())
            for m in range(16):
                pd = psum0.tile([P, 1], F32, space="PSUM", tag="pd")
                for k in range(4):
                    nc.tensor.matmul(
                        pd[:],
                        lhsT=wih_sb[:, k * G4 + m * P:k * G4 + (m + 1) * P],
                        rhs=b1_sb[:, k:k + 1],
                        start=(k == 0), stop=(k == 3))
                nc.vector.tensor_add(out=dcol[:, m:m + 1], in0=dcol[:, m:m + 1],
                                     in1=pd[:])
            psum0_cm.__exit__(None, None, None)

            # ---------------- P1: gather -> x^T -> xin^T -> xg^T ----------
            with tc.tile_pool(name="p1w", bufs=1) as p1w, \
                 tc.tile_pool(name="p1g", bufs=3) as p1g, \
                 tc.tile_pool(name="p1t", bufs=2) as p1t, \
                 tc.tile_pool(name="p1e", bufs=4) as p1e, \
                 tc.tile_pool(name="psum_t", bufs=2, space="PSUM") as psum_t, \
                 tc.tile_pool(name="psum_x", bufs=2, space="PSUM") as psum_x, \
                 tc.tile_pool(name="psum_g", bufs=2, space="PSUM") as psum_g:

                w1_sb = p1w.tile([P, 6 * EMB], BF16)
                nc.sync.dma_start(out=w1_sb[:], in_=w1_in.ap())
                # fold softmax(arch) scale into W1 rows (k-tile k has table
                # index k//2 throughout: 256-row blocks, 128-row tiles)
                for k in range(6):
                    nc.vector.tensor_scalar_mul(
                        w1_sb[:, k * EMB:(k + 1) * EMB],
                        w1_sb[:, k * EMB:(k + 1) * EMB],
                        wbc[:, k // 2:k // 2 + 1])

                for ci in range(n_ch):
                    xT = p1t.tile([P, 6 * CH_TOK], BF16, tag="xT")
                    for ti in range(ch_tile):
                        xg_t = p1g.tile([P, NE * DE], F32, tag="xg_t")
                        for e in range(NE):
                            j = (ci * ch_tile + ti) * NE + e
                            nc.gpsimd.indirect_dma_start(
                                out=xg_t[:, e * DE:(e + 1) * DE],
                                out_offset=None,
                                in_=tables.ap(),
                                in_offset=bass.IndirectOffsetOnAxis(
                                    ap=gidx_sb[:, j:j + 1], axis=0),
                            )
                        for fc in range(6):
                            pt = psum_t.tile([P, P], F32, space="PSUM",
                                             tag="pt")
                            nc.tensor.transpose(
                                out=pt[:],
                                in_=xg_t[:, fc * P:(fc + 1) * P],
                                identity=ident_tr)
                            nc.vector.tensor_copy(
                                out=xT[:, fc * CH_TOK + ti * P:
                                       fc * CH_TOK + (ti + 1) * P],
                                in_=pt[:])

                    xinT = p1t.tile([P, 4 * CH_TOK], BF16, tag="xinT")
                    for m in range(4):
                        px = psum_x.tile([P, CH_TOK], F32, space="PSUM",
                                         tag="px")
                        for k in range(6):
                            nc.tensor.matmul(
                                px[:],
                                lhsT=w1_sb[:, k * EMB + m * P:
                                           k * EMB + (m + 1) * P],
                                rhs=xT[:, k * CH_TOK:(k + 1) * CH_TOK],
                                start=(k == 0), stop=(k == 5))
                        nc.vector.tensor_copy(
                            out=xinT[:, m * CH_TOK:(m + 1) * CH_TOK], in_=px[:])

                    for m in range(16):
                        pg = psum_g.tile([P, CH_TOK], F32, space="PSUM",
                                         tag="pg")
                        for k in range(4):
                            nc.tensor.matmul(
                                pg[:],
                                lhsT=wih_sb[:, k * G4 + m * P:
                                            k * G4 + (m + 1) * P],
                                rhs=xinT[:, k * CH_TOK:(k + 1) * CH_TOK],
                                start=(k == 0), stop=(k == 3))
                        ev = p1e.tile([P, CH_TOK], BF16, tag="ev")
                        nc.vector.tensor_scalar_add(ev[:], pg[:],
                                                    dcol[:, m:m + 1])
                        nc.sync.dma_start(
                            out=xgT.ap()[m * P:(m + 1) * P,
                                         ci * CH_TOK:(ci + 1) * CH_TOK],
                            in_=ev[:])

            # ---------------- P2: LSTM recurrence -------------------------
            # gate tiles (new order): 0-3 g, 4-7 i, 8-11 f, 12-15 o
            with tc.tile_pool(name="hTp", bufs=1) as hTp, \
                 tc.tile_pool(name="stp", bufs=4) as stp, \
                 tc.tile_pool(name="pgg", bufs=1, space="PSUM") as pgg, \
                 tc.tile_pool(name="pff", bufs=1, space="PSUM") as pff, \
                 tc.tile_pool(name="pii", bufs=1, space="PSUM") as pii, \
                 tc.tile_pool(name="pog", bufs=1, space="PSUM") as pog:

                hT = hTp.tile([P, 4 * n_tok], BF16)
                c_sb = hTp.tile([P, HID // 16], F32)   # [128, 32]
                nc.vector.memset(c_sb[:], 0.0)

                BL = B_LOC
                HB = 4 * BL  # 32 cols per gate type
                inv = 1.0 / sc
                for t in range(S):
                    xgt = stp.tile([P, 16 * BL], BF16, tag="xgt")
                    nc.sync.dma_start(
                        out=xgt[:].rearrange("g (gt b) -> g gt b", gt=16),
                        in_=xgT.ap()[:, t * BL:(t + 1) * BL].rearrange(
                            "(gt g) b -> g gt b", g=P))
                    prs = [
                        pool.tile([P, HB], F32, space="PSUM", tag=tg_,
                                  name=tg_)
                        for pool, tg_ in ((pgg, "pr_gg"), (pff, "pr_ff"),
                                          (pii, "pr_ii"), (pog, "pr_o"))]
                    # inject sc*xg via identity matmuls (set has_written)
                    for q in range(4):
                        nc.tensor.matmul(prs[q][:], lhsT=ident_sc,
                                         rhs=xgt[:, q * HB:(q + 1) * HB],
                                         start=True, stop=(t == 0),
                                         skip_group_check=True)
                    if t > 0:
                        for gt in range(16):
                            dst = prs[gt // 4][:, (gt % 4) * BL:
                                               (gt % 4 + 1) * BL]
                            for kt in range(4):
                                rh = hT[:, kt * n_tok + (t - 1) * BL:
                                        kt * n_tok + t * BL]
                                nc.tensor.matmul(
                                    dst,
                                    lhsT=whh_sb[:, kt * G4 + gt * P:
                                                kt * G4 + (gt + 1) * P],
                                    rhs=rh,
                                    start=False, stop=(kt == 3),
                                    skip_group_check=True)

                    tg = stp.tile([P, HB], F32, tag="tg")
                    nc.scalar.activation(tg[:], prs[0][:], AF.Tanh,
                                         scale=inv)
                    sf = stp.tile([P, HB], F32, tag="sf")
                    nc.scalar.activation(sf[:], prs[1][:], AF.Sigmoid,
                                         scale=inv)
                    si = stp.tile([P, HB], F32, tag="si")
                    nc.scalar.activation(si[:], prs[2][:], AF.Sigmoid,
                                         scale=inv)
                    so = stp.tile([P, HB], F32, tag="so")
                    nc.scalar.activation(so[:], prs[3][:], AF.Sigmoid,
                                         scale=inv)
                    fc_ = stp.tile([P, HB], F32, tag="fc_")
                    nc.vector.tensor_tensor(out=fc_[:], in0=sf[:],
                                            in1=c_sb[:], op=ALU.mult)
                    ig_ = stp.tile([P, HB], F32, tag="ig_")
                    nc.vector.tensor_tensor(out=ig_[:], in0=si[:],
                                            in1=tg[:], op=ALU.mult)
                    nc.vector.tensor_add(out=c_sb[:], in0=fc_[:], in1=ig_[:])
                    tc_ = stp.tile([P, HB], F32, tag="tc_")
                    nc.scalar.activation(tc_[:], c_sb[:], AF.Tanh)
                    nc.vector.tensor_tensor(
                        out=hT[:].rearrange("g (kt n) -> g kt n", kt=4)
                            [:, :, t * BL:(t + 1) * BL],
                        in0=so[:].rearrange("g (kt b) -> g kt b", kt=4),
                        in1=tc_[:].rearrange("g (kt b) -> g kt b", kt=4),
                        op=ALU.mult)

                # ------------- P3: Wout partial ---------------------------
                with tc.tile_pool(name="p3", bufs=2) as p3, \
                     tc.tile_pool(name="psum_o", bufs=2, space="PSUM") as psum_o:
                    oT = p3.tile([TAGP2, n_tok], F32, tag="oT")
                    CH_O = CH_TOK
                    for ci in range(n_tok // CH_O):
                        po = psum_o.tile([TAGP2, CH_O], F32, space="PSUM",
                                         tag="po")
                        for kt in range(4):
                            nc.tensor.matmul(
                                po[:],
                                lhsT=wout_sb[:, kt * TAGP2:(kt + 1) * TAGP2],
                                rhs=hT[:, kt * n_tok + ci * CH_O:
                                       kt * n_tok + (ci + 1) * CH_O],
                                start=(kt == 0), stop=(kt == 3))
                        nc.vector.tensor_scalar_add(
                            oT[:, ci * CH_O:(ci + 1) * CH_O], po[:],
                            bout_sb[:, 0:1])
                    nc.sync.dma_start(out=outp.ap(), in_=oT[:])

    nc.compile()
    return nc


# --------------------------------------------------------------------------
_NC_CACHE = {}


def _get_nc(S, V, whh_fp8=True):
    key = (S, V, whh_fp8)
    if key not in _NC_CACHE:
        _NC_CACHE[key] = build_nc(S, V, whh_fp8)
    return _NC_CACHE[key]


def _gate_perm(a, axis=0):
    """Permute pytorch gate blocks [i,f,g,o] -> [g,i,f,o] along axis."""
    blocks = np.split(np.asarray(a), 4, axis=axis)
    return np.concatenate([blocks[g] for g in GPERM], axis=axis)


def _prep_core_inputs(c, token_ids, tables_flat, arch_params, w1, b1,
                      wih_f, whh_f, bih_f, bhh_f, wih_r, whh_r, bih_r, bhh_r,
                      wout, bout, S, V, whh_fp8):
    import ml_dtypes
    d, g = divmod(c, 4)
    ids = token_ids[g * B_LOC:(g + 1) * B_LOC, :]
    if d == 1:
        ids = ids[:, ::-1]
    flat = ids.T.reshape(-1).astype(np.int64)      # s-major [S*B]
    n_tile = flat.shape[0] // P
    base = flat.reshape(n_tile, P)
    gidx = (base[:, :, None] + (np.arange(NE) * V)[None, None, :])
    gidx = gidx.transpose(1, 0, 2).reshape(P, n_tile * NE).astype(np.int32)

    wih = _gate_perm(wih_f if d == 0 else wih_r)
    whh = _gate_perm(whh_f if d == 0 else whh_r)
    bih = _gate_perm(bih_f if d == 0 else bih_r)
    bhh = _gate_perm(bhh_f if d == 0 else bhh_r)

    wihT = np.ascontiguousarray(wih.T)             # [512, 2048]
    whhT = np.ascontiguousarray(whh.T)

    def ktile(a, nk, f):
        # [nk*128, f] -> [128, nk*f] with (k) tiles side by side
        return np.ascontiguousarray(
            a.reshape(nk, P, f).transpose(1, 0, 2).reshape(P, nk * f))

    if whh_fp8:
        whh_host = np.clip(ktile(whhT, 4, G4) * SC, -240.0, 240.0).astype(
            ml_dtypes.float8_e4m3)
    else:
        whh_host = ktile(whhT, 4, G4).astype(ml_dtypes.bfloat16)

    bf = ml_dtypes.bfloat16
    eye = np.eye(P, dtype=np.float32)
    return {
        "tables": tables_flat,
        "identc": ((SC if whh_fp8 else 1.0) * eye).astype(bf),
        "identr": eye,
        "gidx": gidx,
        "arch": arch_params.reshape(1, NE).astype(np.float32),
        "w1": ktile(w1, 6, EMB).astype(bf),
        "wihT": ktile(wihT, 4, G4).astype(bf),
        "whhT": whh_host,
        "wout": ktile(wout[d * HID:(d + 1) * HID, :], 4, TAGP2).astype(bf),
        "b1c": np.ascontiguousarray(b1.reshape(4, P).T).astype(bf),
        "bihg": np.ascontiguousarray(bih.reshape(16, P).T).astype(np.float32),
        "bhhg": np.ascontiguousarray(bhh.reshape(16, P).T).astype(np.float32),
        "boutc": (bout.reshape(TAGP2, 1).astype(np.float32) if d == 0
                  else np.zeros((TAGP2, 1), np.float32)),
    }


def run_cores(token_ids, emb_tables, arch_params, W1, b1,
              Wih_f, Whh_f, bih_f, bhh_f, Wih_r, Whh_r, bih_r, bhh_r,
              Wout, bout, *, whh_fp8=True, trace=False):
    global LAST_EXEC_NS
    B, S = token_ids.shape
    V = emb_tables.shape[1]
    assert B == 32 and emb_tables.shape[0] == NE and emb_tables.shape[2] == DE

    import time as _time
    _t0 = _time.time()
    nc = _get_nc(S, V, whh_fp8)
    _t1 = _time.time()
    tables_flat = np.ascontiguousarray(
        np.asarray(emb_tables, dtype=np.float32).reshape(NE * V, DE))

    args = (np.asarray(token_ids), tables_flat, np.asarray(arch_params),
            np.asarray(W1), np.asarray(b1),
            np.asarray(Wih_f), np.asarray(Whh_f), np.asarray(bih_f),
            np.asarray(bhh_f),
            np.asarray(Wih_r), np.asarray(Whh_r), np.asarray(bih_r),
            np.asarray(bhh_r), np.asarray(Wout), np.asarray(bout))
    in_maps = [
        _prep_core_inputs(c, *args, S, V, whh_fp8) for c in range(N_CORES)
    ]
    _t2 = _time.time()
    res = run_bass_kernel_spmd(nc, in_maps, list(range(N_CORES)), trace=trace)
    LAST_EXEC_NS = res.exec_time_ns
    if os.environ.get("KERNEL_VERBOSE", "0") == "1":
        print(f"[kernel] build {_t1-_t0:.1f}s prep {_t2-_t1:.1f}s "
              f"run {_time.time()-_t2:.1f}s exec_ns={LAST_EXEC_NS}",
              flush=True)

    out = np.zeros((B, S, TAGP2), dtype=np.float32)
    for c in range(N_CORES):
        d, g = divmod(c, 4)
        part = res.results[c]["outp"]                      # [22, S*B_LOC]
        part = np.asarray(part).T.reshape(S, B_LOC, TAGP2)
        if d == 1:
            part = part[::-1]
        out[g * B_LOC:(g + 1) * B_LOC] += part.transpose(1, 0, 2)
    return out


def kernel(token_ids, emb_tables, arch_params, W1, b1,
           Wih_f, Whh_f, bih_f, bhh_f,
           Wih_r, Whh_r, bih_r, bhh_r,
           Wout, bout):
    return run_cores(
        token_ids, emb_tables, arch_params, W1, b1,
        Wih_f, Whh_f, bih_f, bhh_f, Wih_r, Whh_r, bih_r, bhh_r, Wout, bout,
        whh_fp8=os.environ.get("KERNEL_WHH_FP8", "1") == "1",
        trace=os.environ.get("KERNEL_TRACE", "0") == "1",
    )


# revision 14
# speedup vs baseline: 1.2866x; 1.0456x over previous
"""Trainium2 Bass kernel for nn_BERT_LSTM_CRF (embedding MixedOp + Linear +
bidirectional LSTM + output projection), SPMD over 8 NeuronCores.

Sharding: cores 0-3 forward LSTM / cores 4-7 reverse LSTM (reverse is run as a
forward scan over host-flipped sequences); within each direction group the
batch (32) is sharded 4 ways (8 rows per core). Embedding tables are
replicated; each core gathers only the rows for its own 4096 tokens.

Per-core pipeline (all cores run the identical program, only data differs):
  P0  softmax(arch_params) on device; scaled identity matrices; gate bias
      d = bih + bhh + Wih @ b1.
  P1  for each chunk of 512 tokens: indirect-DMA gather of table rows
      -> PE transpose (f32r) -> x^T (bf16); W1 matmul -> xin^T (bf16);
      Wih matmul (+bias) -> xg^T -> DRAM (bf16).
  P2  512-step LSTM recurrence, gates-on-partitions layout, gate order
      [g,i,f,o]: xg injected into PSUM via a scaled-identity matmul
      (start=True), Whh^T matmuls (fp8e4, x16 scale) accumulate on top;
      ACT reads PSUM directly with scale=1/16; i/f/g chain overlaps the
      o-gate matmuls via split PSUM banks.
  P3  Wout half-projection of h^T history -> partial output [22, 4096].

Host reassembles: out[b,s,:] = fwd_part + rev_part (flipped).
"""

import contextlib
import ctypes
import os
import sys
import types

sys.path.insert(0, "/opt/trn_rl_repo")

import numpy as np

import concourse.bacc as bacc
import concourse.bass as bass
import concourse.mybir as mybir
import concourse.tile as tile
from concourse.bass_utils import run_bass_kernel_spmd

F32 = mybir.dt.float32
F32R = mybir.dt.float32r
BF16 = mybir.dt.bfloat16
FP8 = mybir.dt.float8e4
I32 = mybir.dt.int32
AF = mybir.ActivationFunctionType
ALU = mybir.AluOpType

P = 128
DE = 256          # embedding dim per table
NE = 3            # number of tables
EMB = 512         # after W1
HID = 512
G4 = 4 * HID      # 2048 gate dim
TAGP2 = 22
B_LOC = 8         # batch rows per core
N_CORES = 8
SC = 16.0         # fp8 weight scale (ACT un-scales); 1.0 in bf16 mode
# gate order [g, f, i, o] (pytorch blocks are [i, f, g, o])
GPERM = (2, 1, 0, 3)

LAST_EXEC_NS = None


# --------------------------------------------------------------------------
# NTFF profiling shim (antenv.axon_hooks is missing from this image).
def _install_ntff_shim():
    if "antenv.axon_hooks" in sys.modules:
        return

    def _make_hook():
        try:
            lib = ctypes.CDLL("/opt/axon/libaxon_pjrt.so")
        except OSError:
            return None
        if not hasattr(lib, "axon_start_nrt_profile"):
            return None
        lib.axon_start_nrt_profile.argtypes = [
            ctypes.POINTER(ctypes.c_int64),
            ctypes.c_size_t,
        ]
        lib.axon_start_nrt_profile.restype = ctypes.c_int64
        lib.axon_stop_nrt_profile.argtypes = [ctypes.c_char_p]
        lib.axon_stop_nrt_profile.restype = ctypes.c_int64

        @contextlib.contextmanager
        def _hook(output_dir, device_ids):
            import jax

            jax.devices()
            if device_ids:
                ids = (ctypes.c_int64 * len(device_ids))(*device_ids)
                rc = lib.axon_start_nrt_profile(ids, len(device_ids))
            else:
                rc = lib.axon_start_nrt_profile(None, 0)
            if rc != 0:
                raise RuntimeError(f"axon_start_nrt_profile rc={rc}")
            try:
                yield
            finally:
                n = lib.axon_stop_nrt_profile(str(output_dir).encode())
                if n < 0:
                    raise RuntimeError(f"axon_stop_nrt_profile rc={n}")

        return _hook

    mod = types.ModuleType("antenv.axon_hooks")
    mod.get_axon_ntff_profile_hook = _make_hook
    sys.modules["antenv.axon_hooks"] = mod


_install_ntff_shim()


# --------------------------------------------------------------------------
def build_nc(S, V, whh_fp8=True):
    """Build the per-core Bass program. S = sequence length, V = vocab."""
    n_tok = B_LOC * S                    # tokens per core
    n_tile = n_tok // P                  # 128-token tiles
    CH_TOK = 512 if n_tok >= 512 else n_tok   # tokens per P1 chunk
    n_ch = n_tok // CH_TOK               # P1 chunks
    ch_tile = CH_TOK // P                # token-tiles per chunk (4)
    n_gj = n_tile * NE                   # gather calls
    WDT = FP8 if whh_fp8 else BF16
    sc = SC if whh_fp8 else 1.0

    nc = bacc.Bacc("TRN2", target_bir_lowering=False, debug=False,
                   num_devices=N_CORES)

    tables = nc.dram_tensor("tables", [NE * V, DE], F32, kind="ExternalInput")
    gidx_in = nc.dram_tensor("gidx", [P, n_gj], I32, kind="ExternalInput")
    arch_in = nc.dram_tensor("arch", [1, NE], F32, kind="ExternalInput")
    w1_in = nc.dram_tensor("w1", [P, 6 * EMB], BF16, kind="ExternalInput")
    wih_in = nc.dram_tensor("wihT", [P, 4 * G4], BF16, kind="ExternalInput")
    whh_in = nc.dram_tensor("whhT", [P, 4 * G4], WDT, kind="ExternalInput")
    wout_in = nc.dram_tensor("wout", [P, 4 * TAGP2], BF16, kind="ExternalInput")
    b1_in = nc.dram_tensor("b1c", [P, 4], BF16, kind="ExternalInput")
    bih_in = nc.dram_tensor("bihg", [P, 16], F32, kind="ExternalInput")
    bhh_in = nc.dram_tensor("bhhg", [P, 16], F32, kind="ExternalInput")
    bout_in = nc.dram_tensor("boutc", [TAGP2, 1], F32, kind="ExternalInput")
    ident_in = nc.dram_tensor("identc", [P, P], BF16, kind="ExternalInput")
    identr_in = nc.dram_tensor("identr", [P, P], F32, kind="ExternalInput")
    outp = nc.dram_tensor("outp", [TAGP2, n_tok], F32, kind="ExternalOutput")

    # xg^T staging in DRAM: row = gate row (16 tiles x 128), col = s*8+b
    xgT = nc.dram_tensor("xgT", [16 * P, S * B_LOC], BF16, kind="Internal")

    with tile.TileContext(nc) as tc:
        ctx = contextlib.ExitStack()
        with ctx:
            constp = ctx.enter_context(tc.tile_pool(name="constp", bufs=1))
            wper = ctx.enter_context(tc.tile_pool(name="wper", bufs=1))
            psum0_cm = tc.tile_pool(name="psum0", bufs=1, space="PSUM")
            psum0 = psum0_cm.__enter__()

            # ---------------- P0: constants -------------------------------
            gidx_sb = wper.tile([P, n_gj], I32)
            nc.sync.dma_start(out=gidx_sb[:], in_=gidx_in.ap())
            whh_sb = wper.tile([P, 4 * G4], WDT)
            nc.sync.dma_start(out=whh_sb[:], in_=whh_in.ap())
            wout_sb = wper.tile([P, 4 * TAGP2], BF16)
            nc.sync.dma_start(out=wout_sb[:], in_=wout_in.ap())
            bout_sb = wper.tile([TAGP2, 1], F32)
            nc.sync.dma_start(out=bout_sb[:], in_=bout_in.ap())

            # softmax(arch) broadcast to all partitions
            arow = constp.tile([1, NE], F32)
            nc.sync.dma_start(out=arow[:], in_=arch_in.ap())
            erow = constp.tile([1, NE], F32)
            nc.scalar.activation(erow[:], arow[:], AF.Exp)
            srow = constp.tile([1, 1], F32)
            nc.vector.tensor_reduce(out=srow[:], in_=erow[:],
                                    axis=mybir.AxisListType.X, op=ALU.add)
            ones_r = constp.tile([1, P], F32)
            nc.vector.memset(ones_r[:], 1.0)
            rrow = constp.tile([1, 1], F32)
            nc.vector.reciprocal(out=rrow[:], in_=srow[:])
            pe_b = psum0.tile([P, NE], F32, space="PSUM", tag="pe_b")
            nc.tensor.matmul(pe_b[:], lhsT=ones_r[:], rhs=erow[:],
                             start=True, stop=True)
            ps_b = psum0.tile([P, 1], F32, space="PSUM", tag="ps_b")
            nc.tensor.matmul(ps_b[:], lhsT=ones_r[:], rhs=rrow[:],
                             start=True, stop=True)
            ssb = constp.tile([P, 1], F32)
            nc.vector.tensor_copy(out=ssb[:], in_=ps_b[:])
            wbc = constp.tile([P, NE], F32)
            nc.vector.tensor_tensor(out=wbc[:], in0=pe_b[:],
                                    in1=ssb[:].to_broadcast([P, NE]),
                                    op=ALU.mult)

            ident_sc_t = wper.tile([P, P], BF16)     # xg inject (sc * I)
            nc.sync.dma_start(out=ident_sc_t[:], in_=ident_in.ap())
            ident_sc = ident_sc_t[:]
            ident_tr_t = wper.tile([P, P], F32)      # transpose helper
            nc.sync.dma_start(out=ident_tr_t[:], in_=identr_in.ap())
            ident_tr = ident_tr_t[:]

            # gate bias dcol[p, m] = bih + bhh + (Wih @ b1), layout (p, gt)
            b1_sb = constp.tile([P, 4], BF16)
            nc.sync.dma_start(out=b1_sb[:], in_=b1_in.ap())
            bih_sb = constp.tile([P, 16], F32)
            nc.sync.dma_start(out=bih_sb[:], in_=bih_in.ap())
            bhh_sb = constp.tile([P, 16], F32)
            nc.sync.dma_start(out=bhh_sb[:], in_=bhh_in.ap())
            dcol = wper.tile([P, 16], F32)
            nc.vector.tensor_add(out=dcol[:], in0=bih_sb[:], in1=bhh_sb[:])

            wih_sb = wper.tile([P, 4 * G4], BF16)
            nc.sync.dma_start(out=wih_sb[:], in_=wih_in.ap())
            for m in range(16):
                pd = psum0.tile([P, 1], F32, space="PSUM", tag="pd")
                for k in range(4):
                    nc.tensor.matmul(
                        pd[:],
                        lhsT=wih_sb[:, k * G4 + m * P:k * G4 + (m + 1) * P],
                        rhs=b1_sb[:, k:k + 1],
                        start=(k == 0), stop=(k == 3))
                nc.vector.tensor_add(out=dcol[:, m:m + 1], in0=dcol[:, m:m + 1],
                                     in1=pd[:])
            psum0_cm.__exit__(None, None, None)

            # ---------------- P2 pools (opened first so P1 pools can be
            # released mid-loop in LIFO order) --------------------------
            p2_ctx = contextlib.ExitStack()
            hTp = p2_ctx.enter_context(tc.tile_pool(name="hTp", bufs=1))
            stp = p2_ctx.enter_context(tc.tile_pool(name="stp", bufs=4))
            pgg = p2_ctx.enter_context(
                tc.tile_pool(name="pgg", bufs=1, space="PSUM"))
            pff = p2_ctx.enter_context(
                tc.tile_pool(name="pff", bufs=1, space="PSUM"))
            pii = p2_ctx.enter_context(
                tc.tile_pool(name="pii", bufs=1, space="PSUM"))
            pog = p2_ctx.enter_context(
                tc.tile_pool(name="pog", bufs=1, space="PSUM"))

            # ---------------- P1 pools (open through most of P2) ----------
            p1_ctx = contextlib.ExitStack()
            p1w = p1_ctx.enter_context(tc.tile_pool(name="p1w", bufs=1))
            p1g = p1_ctx.enter_context(tc.tile_pool(name="p1g", bufs=4))
            p1t = p1_ctx.enter_context(tc.tile_pool(name="p1t", bufs=2))
            p1e = p1_ctx.enter_context(tc.tile_pool(name="p1e", bufs=4))
            psum_t = p1_ctx.enter_context(
                tc.tile_pool(name="psum_t", bufs=2, space="PSUM"))
            psum_x = p1_ctx.enter_context(
                tc.tile_pool(name="psum_x", bufs=1, space="PSUM"))
            psum_g = p1_ctx.enter_context(
                tc.tile_pool(name="psum_g", bufs=1, space="PSUM"))

            w1_sb = p1w.tile([P, 6 * EMB], BF16)
            nc.sync.dma_start(out=w1_sb[:], in_=w1_in.ap())
            # fold softmax(arch) scale into W1 rows (k-tile k has table
            # index k//2 throughout: 256-row blocks, 128-row tiles)
            for k in range(6):
                nc.vector.tensor_scalar_mul(
                    w1_sb[:, k * EMB:(k + 1) * EMB],
                    w1_sb[:, k * EMB:(k + 1) * EMB],
                    wbc[:, k // 2:k // 2 + 1])

            def p1_chunk(ci):
                """Emit P1 for chunk ci as a generator of small atoms, so
                chunk ci+2's work can fill PE-idle tails of recurrence
                steps 64*ci..64*ci+63."""
                xT = p1t.tile([P, 6 * CH_TOK], BF16, tag="xT", name="xT")
                for ti in range(ch_tile):
                    xg_t = p1g.tile([P, NE * DE], F32, tag="xg_t",
                                    name="xg_t")
                    for e in range(NE):
                        j = (ci * ch_tile + ti) * NE + e
                        nc.gpsimd.indirect_dma_start(
                            out=xg_t[:, e * DE:(e + 1) * DE],
                            out_offset=None,
                            in_=tables.ap(),
                            in_offset=bass.IndirectOffsetOnAxis(
                                ap=gidx_sb[:, j:j + 1], axis=0),
                        )
                        yield
                    for h3 in range(0, 6, 3):
                        for fcc in range(h3, h3 + 3):
                            pt = psum_t.tile([P, P], F32, space="PSUM",
                                             tag="pt", name="pt")
                            nc.tensor.transpose(
                                out=pt[:],
                                in_=xg_t[:, fcc * P:(fcc + 1) * P],
                                identity=ident_tr)
                            nc.vector.tensor_copy(
                                out=xT[:, fcc * CH_TOK + ti * P:
                                       fcc * CH_TOK + (ti + 1) * P],
                                in_=pt[:])
                        yield
                xinT = p1t.tile([P, 4 * CH_TOK], BF16, tag="xinT",
                                name="xinT")
                for m in range(4):
                    px = psum_x.tile([P, CH_TOK], F32, space="PSUM",
                                     tag="px", name="px")
                    for k in range(6):
                        nc.tensor.matmul(
                            px[:],
                            lhsT=w1_sb[:, k * EMB + m * P:
                                       k * EMB + (m + 1) * P],
                            rhs=xT[:, k * CH_TOK:(k + 1) * CH_TOK],
                            start=(k == 0), stop=(k == 5),
                            skip_group_check=True)
                        if k == 2:
                            yield
                    nc.vector.tensor_copy(
                        out=xinT[:, m * CH_TOK:(m + 1) * CH_TOK], in_=px[:])
                    yield
                for m in range(16):
                    pg = psum_g.tile([P, CH_TOK], F32, space="PSUM",
                                     tag="pg", name="pg")
                    for k in range(4):
                        nc.tensor.matmul(
                            pg[:],
                            lhsT=wih_sb[:, k * G4 + m * P:
                                        k * G4 + (m + 1) * P],
                            rhs=xinT[:, k * CH_TOK:(k + 1) * CH_TOK],
                            start=(k == 0), stop=(k == 3),
                            skip_group_check=True)
                        if k == 1:
                            yield
                    ev = p1e.tile([P, CH_TOK], BF16, tag="ev", name="ev")
                    nc.vector.tensor_scalar_add(ev[:], pg[:],
                                                dcol[:, m:m + 1])
                    nc.sync.dma_start(
                        out=xgT.ap()[m * P:(m + 1) * P,
                                     ci * CH_TOK:(ci + 1) * CH_TOK],
                        in_=ev[:])
                    yield

            # prologue: first two chunks fully (recurrence needs them at
            # steps 0 and 64; interleaved production starts at chunk 2)
            for ci in range(min(2, n_ch)):
                for _ in p1_chunk(ci):
                    pass

            # ---------------- P2: LSTM recurrence -------------------------
            # gate tiles (order): 0-3 g, 4-7 f, 8-11 i, 12-15 o
            with p2_ctx:
                hT = hTp.tile([P, 4 * n_tok], BF16)
                c_sb = hTp.tile([P, HID // 16], F32)   # [128, 32]
                nc.vector.memset(c_sb[:], 0.0)

                BL = B_LOC
                HB = 4 * BL  # 32 cols per gate type
                inv = 1.0 / sc
                S_CH = S // n_ch                       # steps per chunk (64)
                PF = 3                                 # xgt prefetch depth

                xgt_q = []

                def issue_xgt(tt):
                    xgt = stp.tile([P, 16 * BL], BF16, tag="xgt", name="xgt")
                    nc.sync.dma_start(
                        out=xgt[:].rearrange("g (gt b) -> g gt b", gt=16),
                        in_=xgT.ap()[:, tt * BL:(tt + 1) * BL].rearrange(
                            "(gt g) b -> g gt b", g=P))
                    xgt_q.append(xgt)

                for tt in range(min(PF, S)):
                    issue_xgt(tt)

                gen = None           # active P1 chunk generator
                p3_pools = None      # set once P1 pools are closed
                p3_cur = None        # active P3 chunk atom iterator
                p3_ci = 0

                def p3_chunk(ci):
                    po = psum_o.tile([TAGP2, CH_TOK], F32, space="PSUM",
                                     tag="po", name="po")
                    for kt in range(4):
                        nc.tensor.matmul(
                            po[:],
                            lhsT=wout_sb[:, kt * TAGP2:(kt + 1) * TAGP2],
                            rhs=hT[:, kt * n_tok + ci * CH_TOK:
                                   kt * n_tok + (ci + 1) * CH_TOK],
                            start=(kt == 0), stop=(kt == 3),
                            skip_group_check=True)
                        if kt == 1:
                            yield
                    nc.vector.tensor_scalar_add(
                        oT[:, ci * CH_TOK:(ci + 1) * CH_TOK], po[:],
                        bout_sb[:, 0:1])
                    yield

                for t in range(S):
                    if t % S_CH == 0:
                        if gen is not None:
                            for _ in gen:
                                pass
                        nci = t // S_CH + 2
                        gen = p1_chunk(nci) if nci < n_ch else None
                        if nci == n_ch:
                            # all of P1 emitted: free its PSUM banks and
                            # start draining P3 into the remaining tails
                            p1_ctx.close()
                            psum_o = p2_ctx.enter_context(
                                tc.tile_pool(name="psum_o", bufs=2,
                                             space="PSUM"))
                            p3p = p2_ctx.enter_context(
                                tc.tile_pool(name="p3", bufs=1))
                            oT = p3p.tile([TAGP2, n_tok], F32)
                            p3_pools = True

                    xgt = xgt_q.pop(0)
                    prs = [
                        pool.tile([P, HB], F32, space="PSUM", tag=tg_,
                                  name=tg_)
                        for pool, tg_ in ((pgg, "pr_gg"), (pff, "pr_ff"),
                                          (pii, "pr_ii"), (pog, "pr_o"))]
                    # inject sc*xg via identity matmuls (set has_written)
                    for q in range(4):
                        nc.tensor.matmul(prs[q][:], lhsT=ident_sc,
                                         rhs=xgt[:, q * HB:(q + 1) * HB],
                                         start=True, stop=(t == 0),
                                         skip_group_check=True)
                    if t > 0:
                        for gt in range(16):
                            dst = prs[gt // 4][:, (gt % 4) * BL:
                                               (gt % 4 + 1) * BL]
                            for kt in range(4):
                                rh = hT[:, kt * n_tok + (t - 1) * BL:
                                        kt * n_tok + t * BL]
                                nc.tensor.matmul(
                                    dst,
                                    lhsT=whh_sb[:, kt * G4 + gt * P:
                                                kt * G4 + (gt + 1) * P],
                                    rhs=rh,
                                    start=False, stop=(kt == 3),
                                    skip_group_check=True)

                    tg = stp.tile([P, HB], F32, tag="tg")
                    nc.scalar.activation(tg[:], prs[0][:], AF.Tanh,
                                         scale=inv)
                    sf = stp.tile([P, HB], F32, tag="sf")
                    nc.scalar.activation(sf[:], prs[1][:], AF.Sigmoid,
                                         scale=inv)
                    si = stp.tile([P, HB], F32, tag="si")
                    nc.scalar.activation(si[:], prs[2][:], AF.Sigmoid,
                                         scale=inv)
                    so = stp.tile([P, HB], F32, tag="so")
                    nc.scalar.activation(so[:], prs[3][:], AF.Sigmoid,
                                         scale=inv)
                    fc_ = stp.tile([P, HB], F32, tag="fc_")
                    nc.vector.tensor_tensor(out=fc_[:], in0=sf[:],
                                            in1=c_sb[:], op=ALU.mult)
                    ig_ = stp.tile([P, HB], F32, tag="ig_")
                    nc.vector.tensor_tensor(out=ig_[:], in0=si[:],
                                            in1=tg[:], op=ALU.mult)
                    nc.vector.tensor_add(out=c_sb[:], in0=fc_[:], in1=ig_[:])
                    tc_ = stp.tile([P, HB], F32, tag="tc_")
                    nc.scalar.activation(tc_[:], c_sb[:], AF.Tanh)
                    nc.vector.tensor_tensor(
                        out=hT[:].rearrange("g (kt n) -> g kt n", kt=4)
                            [:, :, t * BL:(t + 1) * BL],
                        in0=so[:].rearrange("g (kt b) -> g kt b", kt=4),
                        in1=tc_[:].rearrange("g (kt b) -> g kt b", kt=4),
                        op=ALU.mult)

                    if t + PF < S:
                        issue_xgt(t + PF)

                    # fill this step's PE-idle tail with P1 or P3 work
                    if gen is not None:
                        next(gen, None)
                    elif p3_pools:
                        if p3_cur is None and p3_ci < n_ch and \
                                S_CH * (p3_ci + 1) <= t:
                            p3_cur = p3_chunk(p3_ci)
                            p3_ci += 1
                        if p3_cur is not None:
                            if next(p3_cur, StopIteration) is StopIteration:
                                p3_cur = None

                # ------------- P3 epilogue --------------------------------
                if p3_cur is not None:
                    for _ in p3_cur:
                        pass
                while p3_ci < n_ch:
                    for _ in p3_chunk(p3_ci):
                        pass
                    p3_ci += 1
                nc.sync.dma_start(out=outp.ap(), in_=oT[:])

    nc.compile()
    return nc


# --------------------------------------------------------------------------
_NC_CACHE = {}


def _get_nc(S, V, whh_fp8=True):
    key = (S, V, whh_fp8)
    if key not in _NC_CACHE:
        _NC_CACHE[key] = build_nc(S, V, whh_fp8)
    return _NC_CACHE[key]


def _gate_perm(a, axis=0):
    """Permute pytorch gate blocks [i,f,g,o] -> [g,i,f,o] along axis."""
    blocks = np.split(np.asarray(a), 4, axis=axis)
    return np.concatenate([blocks[g] for g in GPERM], axis=axis)


def _prep_core_inputs(c, token_ids, tables_flat, arch_params, w1, b1,
                      wih_f, whh_f, bih_f, bhh_f, wih_r, whh_r, bih_r, bhh_r,
                      wout, bout, S, V, whh_fp8):
    import ml_dtypes
    d, g = divmod(c, 4)
    ids = token_ids[g * B_LOC:(g + 1) * B_LOC, :]
    if d == 1:
        ids = ids[:, ::-1]
    flat = ids.T.reshape(-1).astype(np.int64)      # s-major [S*B]
    n_tile = flat.shape[0] // P
    base = flat.reshape(n_tile, P)
    gidx = (base[:, :, None] + (np.arange(NE) * V)[None, None, :])
    gidx = gidx.transpose(1, 0, 2).reshape(P, n_tile * NE).astype(np.int32)

    wih = _gate_perm(wih_f if d == 0 else wih_r)
    whh = _gate_perm(whh_f if d == 0 else whh_r)
    bih = _gate_perm(bih_f if d == 0 else bih_r)
    bhh = _gate_perm(bhh_f if d == 0 else bhh_r)

    wihT = np.ascontiguousarray(wih.T)             # [512, 2048]
    whhT = np.ascontiguousarray(whh.T)

    def ktile(a, nk, f):
        # [nk*128, f] -> [128, nk*f] with (k) tiles side by side
        return np.ascontiguousarray(
            a.reshape(nk, P, f).transpose(1, 0, 2).reshape(P, nk * f))

    if whh_fp8:
        whh_host = np.clip(ktile(whhT, 4, G4) * SC, -240.0, 240.0).astype(
            ml_dtypes.float8_e4m3)
    else:
        whh_host = ktile(whhT, 4, G4).astype(ml_dtypes.bfloat16)

    bf = ml_dtypes.bfloat16
    eye = np.eye(P, dtype=np.float32)
    return {
        "tables": tables_flat,
        "identc": ((SC if whh_fp8 else 1.0) * eye).astype(bf),
        "identr": eye,
        "gidx": gidx,
        "arch": arch_params.reshape(1, NE).astype(np.float32),
        "w1": ktile(w1, 6, EMB).astype(bf),
        "wihT": ktile(wihT, 4, G4).astype(bf),
        "whhT": whh_host,
        "wout": ktile(wout[d * HID:(d + 1) * HID, :], 4, TAGP2).astype(bf),
        "b1c": np.ascontiguousarray(b1.reshape(4, P).T).astype(bf),
        "bihg": np.ascontiguousarray(bih.reshape(16, P).T).astype(np.float32),
        "bhhg": np.ascontiguousarray(bhh.reshape(16, P).T).astype(np.float32),
        "boutc": (bout.reshape(TAGP2, 1).astype(np.float32) if d == 0
                  else np.zeros((TAGP2, 1), np.float32)),
    }


def run_cores(token_ids, emb_tables, arch_params, W1, b1,
              Wih_f, Whh_f, bih_f, bhh_f, Wih_r, Whh_r, bih_r, bhh_r,
              Wout, bout, *, whh_fp8=True, trace=False):
    global LAST_EXEC_NS
    B, S = token_ids.shape
    V = emb_tables.shape[1]
    assert B == 32 and emb_tables.shape[0] == NE and emb_tables.shape[2] == DE

    import time as _time
    _t0 = _time.time()
    nc = _get_nc(S, V, whh_fp8)
    _t1 = _time.time()
    tables_flat = np.ascontiguousarray(
        np.asarray(emb_tables, dtype=np.float32).reshape(NE * V, DE))

    args = (np.asarray(token_ids), tables_flat, np.asarray(arch_params),
            np.asarray(W1), np.asarray(b1),
            np.asarray(Wih_f), np.asarray(Whh_f), np.asarray(bih_f),
            np.asarray(bhh_f),
            np.asarray(Wih_r), np.asarray(Whh_r), np.asarray(bih_r),
            np.asarray(bhh_r), np.asarray(Wout), np.asarray(bout))
    in_maps = [
        _prep_core_inputs(c, *args, S, V, whh_fp8) for c in range(N_CORES)
    ]
    _t2 = _time.time()
    res = run_bass_kernel_spmd(nc, in_maps, list(range(N_CORES)), trace=trace)
    LAST_EXEC_NS = res.exec_time_ns
    if os.environ.get("KERNEL_VERBOSE", "0") == "1":
        print(f"[kernel] build {_t1-_t0:.1f}s prep {_t2-_t1:.1f}s "
              f"run {_time.time()-_t2:.1f}s exec_ns={LAST_EXEC_NS}",
              flush=True)

    out = np.zeros((B, S, TAGP2), dtype=np.float32)
    for c in range(N_CORES):
        d, g = divmod(c, 4)
        part = res.results[c]["outp"]                      # [22, S*B_LOC]
        part = np.asarray(part).T.reshape(S, B_LOC, TAGP2)
        if d == 1:
            part = part[::-1]
        out[g * B_LOC:(g + 1) * B_LOC] += part.transpose(1, 0, 2)
    return out


def kernel(token_ids, emb_tables, arch_params, W1, b1,
           Wih_f, Whh_f, bih_f, bhh_f,
           Wih_r, Whh_r, bih_r, bhh_r,
           Wout, bout):
    return run_cores(
        token_ids, emb_tables, arch_params, W1, b1,
        Wih_f, Whh_f, bih_f, bhh_f, Wih_r, Whh_r, bih_r, bhh_r, Wout, bout,
        whh_fp8=os.environ.get("KERNEL_WHH_FP8", "1") == "1",
        trace=os.environ.get("KERNEL_TRACE", "0") == "1",
    )
